# revision 22
# baseline (speedup 1.0000x reference)
"""Trainium2 Bass kernel for causal self-attention (dense transformer block attn).

Reference computation (per batch b):
    qkv = x @ W_attn + b_attn ; split into per-head Q, K, V (16 heads, hs=64)
    att = softmax(mask(Q K^T / sqrt(hs))) ; y = att @ V ; out = y @ W_proj + b_proj

Sharding (8 cores): data parallel on B (2) x tensor parallel on head groups
(4 groups of 4 heads, Megatron-style column/row split of W_attn / W_proj).
Each core computes a partial out^T [1024, 2048] (f32); host sums the 4 partials
per batch, adds the output bias and transposes.

Bias simplifications (exact):
  - K bias is dropped: with Q keeping its bias, scores change only by a
    per-query constant along k, which softmax cancels exactly.
  - V bias is folded into the host-side output bias: softmax rows sum to 1,
    so y = P(V0 + b_v) = P V0 + b_v, and b_v @ W_proj is added on the host.

Core kernel layout notes:
  - Everything on-chip is transposed: x^T, qkv^T ([feature, T]); scores are
    computed as S^T = K Q^T with k-positions on partitions so the PV matmul
    needs no transposes (P^T moving, V natural stationary).
  - Scores/exp/PV are emitted GROUP-MAJOR: group G covers query block
    [512G, 512G+512).  Within a group, per k-chunk j: a scores matmul pair
    (both heads packed via PE row groups), one wide exp on ACT, and the PV
    accumulation matmuls for the same group lagged two chunks behind.  ACT
    (the serial exp resource, ~70+us) is the pacing engine; filler matmuls
    (QKV chunks, V transposes, projection) keep the PE busy in the slack.
  - Softmax denominator: the PV stationary is [V | ones] (or [ones | V]) so
    the opposite 64 partitions of the PV psum hold 64 copies of sum_k P;
    reciprocal_approx_fast runs on those rows in place, one sbuf->sbuf DMA
    shifts the reciprocal rows onto the y partitions, one DVE multiply
    normalizes.  No DRAM bounce.
  - ~8 junk matmuls at kernel start warm the PE HAM clock gate (1.2->2.4GHz)
    while the input DMAs stream in.
"""

import numpy as np
import ml_dtypes

import concourse.bass as bass
import concourse.tile as tile
import concourse.mybir as mybir
from concourse import bacc
from concourse.bass_utils import run_bass_kernel_spmd

BF16 = mybir.dt.bfloat16
F32 = mybir.dt.float32
AF = mybir.ActivationFunctionType

T = 2048          # sequence length
C = 1024          # model dim
HPC = 4           # heads per core
HS = 64           # head size
NF = 3 * HPC * HS  # per-core qkv features (768)
N_CORES = 8
QB = 512          # q block (psum bank of f32)

bf16 = ml_dtypes.bfloat16


def build_kernel():
    nc = bacc.Bacc("TRN2", target_bir_lowering=False, debug=False)

    xT = nc.dram_tensor("xT", [128, 4, 8, QB], BF16, kind="ExternalInput").ap()
    W = nc.dram_tensor("W", [6, 128, 8, 128], BF16, kind="ExternalInput").ap()
    bcols = nc.dram_tensor("bcols", [128, 2], F32, kind="ExternalInput").ap()
    Wp = nc.dram_tensor("Wp", [HPC * HS, C], BF16, kind="ExternalInput").ap()
    mask = nc.dram_tensor("mask", [128, 128], BF16, kind="ExternalInput").ap()
    ident = nc.dram_tensor("ident", [128, 128], BF16, kind="ExternalInput").ap()
    outT = nc.dram_tensor("outT", [C, T], F32, kind="ExternalOutput").ap()

    with tile.TileContext(nc) as tc:
        _emit(nc, tc, xT, W, bcols, Wp, mask, ident, outT)
    nc.compile()
    return nc


def _emit(nc, tc, xT, W, bcols, Wp, mask, ident, outT, debug_out=None):
    from contextlib import ExitStack
    from collections import deque

    ctx = ExitStack()
    consts = ctx.enter_context(tc.tile_pool(name="consts", bufs=1))
    pt_pool = ctx.enter_context(tc.tile_pool(name="pt", bufs=1))
    rt_pool = ctx.enter_context(tc.tile_pool(name="rt", bufs=2))
    ob_pool = ctx.enter_context(tc.tile_pool(name="ob", bufs=2))
    ps_sc = ctx.enter_context(tc.tile_pool(name="ps_sc", bufs=2, space="PSUM"))
    ps_sm = ctx.enter_context(tc.tile_pool(name="ps_sm", bufs=2, space="PSUM"))
    ps_pv = ctx.enter_context(tc.tile_pool(name="ps_pv", bufs=2, space="PSUM"))

    # ---------------- constant / input loads ----------------
    # nf-major / qb-major layouts so each input DMA writes one CONTIGUOUS
    # slice (strided write regions confuse subtile dependency tracking)
    xT_t = consts.tile([128, 4, 8, QB], BF16, tag="xT", name="xT_t")
    W_t = consts.tile([128, 6, 8, 128], BF16, tag="W", name="W_t")
    b_t = consts.tile([128, 2], F32, tag="b", name="b_t")
    Wp_t = consts.tile([128, 2, C], BF16, tag="Wp", name="Wp_t")
    mask_t = consts.tile([128, 128], BF16, tag="mask", name="mask_t")
    id_t = consts.tile([128, 128], BF16, tag="ident", name="id_t")
    junk_in = consts.tile([128, QB], BF16, tag="junk", name="junk_in")

    qkvT = consts.tile([128, 6, T], BF16, tag="qkvT", name="qkvT")
    # vnat[p, pair, j, hl, col]: PV stationary tiles. hl=0: [V | ones],
    # hl=1: [ones | V] so y lands on the partitions matching yT layout.
    vnat = consts.tile([128, 2, 16, 2, 128], BF16, tag="vnat", name="vnat")
    yT = consts.tile([128, 2, T], BF16, tag="yT", name="yT")

    warm = consts.tile([128, 8], F32, tag="warm", name="warm")
    nc.vector.memset(junk_in, 0.0)   # first DVE op: unblocks the PE warmup
    nc.vector.memset(warm, 0.0)
    nc.vector.memset(vnat[:, :, :, 0, 64:128], 1.0)
    nc.vector.memset(vnat[:, :, :, 1, 0:64], 1.0)

    # dependency-free matmuls: warm the PE HAM clock gate during the input
    # DMA lead-in (and available as tail filler)
    def junk_mm():
        ps = ps_sm.tile([128, QB], F32, tag="sm", name="ps_junk")
        nc.tensor.matmul(ps, lhsT=junk_in[:, 0:128], rhs=junk_in,
                         start=True, stop=True, skip_group_check=True)

    with nc.named_scope("warmup"):
        for _ in range(10):
            junk_mm()

    # input DMAs: x qb0 + W chunks on the sync queue, x qb1-3 on the scalar
    # queue, small constants via gpsimd.  All transfers are contiguous per
    # partition (host packs x/W accordingly) so each issue is ~0.6us.
    # Ordered so the first QKV chunk can start at ~4us: x qb0, W0, W2 first.
    nc.sync.dma_start(out=xT_t[:, 0], in_=xT[:, 0])
    for nf in (0, 2, 4, 1, 3, 5):
        nc.sync.dma_start(out=W_t[:, nf], in_=W[nf])
    for qb4 in range(1, 4):
        nc.scalar.dma_start(out=xT_t[:, qb4], in_=xT[:, qb4])
    nc.gpsimd.dma_start(out=b_t, in_=bcols)
    nc.gpsimd.dma_start(out=mask_t, in_=mask)
    nc.gpsimd.dma_start(out=id_t, in_=ident)
    nc.gpsimd.dma_start(out=Wp_t, in_=Wp.rearrange("(k p) n -> p k n", p=128))

    # warm the ACT exp table (~2.7us) during the DMA lead-in (after the
    # scalar-queue DMA issues so they are not delayed by the table load)
    nc.scalar.activation(warm, warm, AF.Exp, scale=1.0)

    # ---------------- phase helpers ----------------
    def qkv_part(nf, qb4):
        # qkv^T[nf*128:(nf+1)*128, qb] = (x @ W[:, cols])^T; Q gets its bias
        ps = ps_sm.tile([128, QB], F32, tag="sm", name="ps_qkv")
        for c in range(8):
            nc.tensor.matmul(
                ps,
                lhsT=W_t[:, nf, c, :],
                rhs=xT_t[:, qb4, c, :],
                start=(c == 0),
                stop=(c == 7),
            )
        dst = qkvT[:, nf, qb4 * QB:(qb4 + 1) * QB]
        if nf < 2:
            nc.vector.tensor_scalar_add(dst, ps, b_t[:, nf:nf + 1])
        else:
            nc.vector.tensor_copy(dst, ps)

    def vtrans(p, j):
        # V^T chunk j (qkvT[:, 4+p]) -> natural V in vnat[:, p, j]
        pst = ps_sm.tile([128, 128], BF16, tag="sm", name="ps_vt")
        nc.tensor.transpose(pst, qkvT[:, 4 + p, j * 128:(j + 1) * 128], id_t)
        # single strided copy: psum cols [0:64|64:128] -> vnat
        # [j, 0, 0:64] and [j, 1, 64:128]
        v0 = vnat[:, p, j, 0, 0:64]
        dst = bass.AP(tensor=v0.tensor, offset=v0.offset,
                      ap=[v0.ap[0], [192, 2], [1, 64]])
        s0 = pst[:, 0:64]
        src = bass.AP(tensor=s0.tensor, offset=s0.offset,
                      ap=[s0.ap[0], [64, 2], [1, 64]])
        nc.vector.tensor_copy(dst, src)

    pt_tiles = {}

    def pt_get(p, j):
        if (p, j) not in pt_tiles:
            wj = T - 128 * j
            pt_tiles[(p, j)] = pt_pool.tile(
                [128, 2, wj], BF16, tag=f"pt{j}", name=f"pt_{p}_{j}",
                bufs=2 if j < 2 else 1)
        return pt_tiles[(p, j)]

    def s_piece(p, j, g):
        # scores^T + exp for pair p, k-chunk j, query group g (cols
        # [qlo, qhi) of the full sequence); both heads in one go.
        qlo = max(128 * j, QB * g)
        qhi = QB * (g + 1)
        seg = qhi - qlo
        pt = pt_get(p, j)
        ps = ps_sc.tile([128, 2, QB], F32, tag="sc", name="ps_s")
        for hl in range(2):
            nc.tensor.matmul(
                ps[:, hl, 0:seg],
                lhsT=qkvT[64 * hl:64 * hl + 64, 2 + p, j * 128:(j + 1) * 128],
                rhs=qkvT[64 * hl:64 * hl + 64, p, qlo:qhi],
                start=True,
                stop=True,
            )
        nc.scalar.activation(
            pt[:, :, (qlo - 128 * j):(qhi - 128 * j)],
            ps[:, :, 0:seg],
            AF.Exp,
            scale=0.125,
        )
        if j >= 4 * g:
            # diagonal chunk: zero the q < k upper triangle (both heads via a
            # broadcast AP over the head dim)
            mb = bass.AP(tensor=mask_t.tensor, offset=mask_t.offset,
                         ap=[mask_t.ap[0], [0, 2], [1, 128]])
            nc.vector.tensor_mul(pt[:, :, 0:128], pt[:, :, 0:128], mb)

    pv_ps = {}

    def pv_mm(p, hl, g, jp):
        # one PV accumulation matmul for head (p, hl), query group g, chunk jp
        if (p, hl, g) not in pv_ps:
            pv_ps[(p, hl, g)] = ps_pv.tile(
                [128, QB], F32, tag="pv", name=f"ps_pv{p}{hl}{g}")
        ps = pv_ps[(p, hl, g)]
        pt = pt_tiles[(p, jp)]
        qlo = max(QB * g, 128 * jp)
        qhi = QB * g + QB
        last = 4 * g + 3
        nc.tensor.matmul(
            ps[:, (qlo - QB * g):(qhi - QB * g)],
            lhsT=vnat[:, p, jp, hl, :],
            rhs=pt[:, hl, (qlo - 128 * jp):(qhi - 128 * jp)],
            start=(jp == 0),
            stop=(jp == last),
        )

    pending_mul = []
    # DRAM scratch for the reciprocal reshape bounce: [unit, pre/post, 512]
    scr = nc.dram_tensor("pv_scr", [16, 2, QB], F32).ap()

    def pv_evac_a(p, hl, g):
        # psum rows: y at ysl, 64 copies of the softmax denominator at dsl.
        # One fast copy frees the psum bank (so the next group's PV matmuls
        # are never blocked).  InstReciprocal cost scales with FREE size only,
        # so bounce one denominator row through DRAM to reshape
        # [1,512] -> [128,4], recip there (~170ns), and bounce back with a
        # partition-broadcast onto the y rows.  The normalize multiply is
        # deferred (pending_mul) so the in-order DVE queue never blocks on
        # the DMA roundtrip.
        ps = pv_ps.pop((p, hl, g))
        ysl = slice(64 * hl, 64 * hl + 64)
        dsl = slice(64 - 64 * hl, 128 - 64 * hl)
        uid = (p * 2 + hl) * 4 + g
        sb = rt_pool.tile([128, QB], F32, tag="sb", name="sb")
        nc.vector.tensor_copy(sb, ps)
        nc.sync.dma_start(out=scr[uid, 0, :], in_=sb[dsl.start:dsl.start + 1, :])
        rtp = rt_pool.tile([128, 4], F32, tag="rtp", name="rtp")
        nc.sync.dma_start(out=rtp, in_=scr[uid, 0, :].rearrange("(a f) -> a f", f=4))
        rtq = rt_pool.tile([128, 4], F32, tag="rtq", name="rtq")
        nc.vector.reciprocal(rtq, rtp)
        nc.sync.dma_start(out=scr[uid, 1, :].rearrange("(a f) -> a f", f=4), in_=rtq)
        s1 = scr[uid, 1, :]
        bcast = bass.AP(tensor=s1.tensor, offset=s1.offset, ap=[[0, 64], [1, QB]])
        rt2 = rt_pool.tile([128, QB], F32, tag="rt2", name="rt2")
        nc.sync.dma_start(out=rt2[ysl, :], in_=bcast)
        pending_mul.append((sb, rt2, ysl, p, g))

    def flush_mul():
        while pending_mul:
            sb, rt2, ysl, p, g = pending_mul.pop(0)
            nc.vector.tensor_mul(
                yT[ysl, p, g * QB:(g + 1) * QB], sb[ysl, :], rt2[ysl, :])

    outT_v = outT.rearrange("(n p) t -> p n t", p=128)
    ob_tiles = {}

    def proj_nf(g, nf):
        # final projection for query group g, output feature chunk nf
        qsl = slice(g * QB, (g + 1) * QB)
        nf2 = nf // 2
        if (g, nf2) not in ob_tiles:
            ob_tiles[(g, nf2)] = ob_pool.tile(
                [128, 2, QB], F32, tag="ob", name="ob")
        ob = ob_tiles[(g, nf2)]
        ps = ps_sm.tile([128, QB], F32, tag="sm", name="ps_o")
        for kc in range(2):
            nc.tensor.matmul(
                ps,
                lhsT=Wp_t[:, kc, nf * 128:(nf + 1) * 128],
                rhs=yT[:, kc, qsl],
                start=(kc == 0),
                stop=(kc == 1),
            )
        nc.vector.tensor_copy(ob[:, nf % 2, :], ps)
        if nf % 2 == 1:
            del ob_tiles[(g, nf2)]
            nc.sync.dma_start(out=outT_v[:, nf2 * 2:nf2 * 2 + 2, qsl], in_=ob)

    # ---------------- filler machinery ----------------
    # Filler units are (cost_ns, kind, key, fn).  The debt counter releases
    # them as ACT-time accumulates; need_*() force-pops for dependencies.
    filler = deque()
    done_keys = set()
    state = {"debt": 0.0}

    def add_fill(cost, kind, key, fn):
        filler.append((cost, kind, key, fn))

    def pop_one():
        cost, kind, key, fn = filler.popleft()
        fn()
        done_keys.add((kind, key))
        state["debt"] -= cost

    def fill():
        # cap pops per slot so a burst of filler never makes the next
        # scores piece (and its exp) late
        n = 0
        while filler and state["debt"] >= filler[0][0] and n < 2:
            pop_one()
            n += 1

    def need(kind, key):
        while (kind, key) not in done_keys:
            assert filler, f"filler exhausted needing {kind} {key}"
            pop_one()

    def piece_budget(seg):
        # PE is the overall bottleneck (~115us busy vs ACT ~80us): spread
        # the ~70us of filler work evenly over the 80 scores pieces instead
        # of chasing the ACT-vs-PE differential.
        return 1100.0

    # ---------------- emission schedule ----------------
    with nc.named_scope("lead"):
        for nf, qb4 in ((0, 0), (2, 0), (0, 1), (2, 1)):
            qkv_part(nf, qb4)
            done_keys.add(("qkv", (nf, qb4)))

    # filler for the pair-0 phase; order respects data arrival (x qb DMAs)
    # and downstream needs (nf4 early for vtrans(0), Q1/K1/V1 for pair 1)
    for nf, qb4 in ((4, 0), (0, 2), (2, 2), (4, 1), (0, 3), (2, 3),
                    (4, 2), (4, 3), (1, 0), (3, 0), (1, 1), (3, 1),
                    (5, 0), (5, 1), (1, 2), (3, 2), (1, 3), (3, 3),
                    (5, 2), (5, 3)):
        add_fill(2200, "qkv", (nf, qb4),
                 (lambda nf=nf, qb4=qb4: qkv_part(nf, qb4)))
        if nf >= 4:
            p, jb = nf - 4, qb4 * 4
            for j in range(jb, jb + 4):
                add_fill(200, "vt", (p, j), (lambda p=p, j=j: vtrans(p, j)))

    def run_pair(p, scope):
        for g in range(4):
            with nc.named_scope(f"{scope}g{g}"):
                last = 4 * g + 3
                for j in range(last + 1):
                    need("qkv", (p, g))          # Q block for this group
                    need("qkv", (2 + p, j // 4))  # K block for this chunk
                    s_piece(p, j, g)
                    state["debt"] += piece_budget(QB * (g + 1) - max(128 * j, QB * g))
                    if j == 1:
                        flush_mul()              # previous group's normalizes
                    if j >= 2:
                        need("vt", (p, j - 2))
                        pv_mm(p, 0, g, j - 2)
                        pv_mm(p, 1, g, j - 2)
                    fill()
                for jp in (last - 1, last):
                    need("vt", (p, jp))
                    pv_mm(p, 0, g, jp)
                    pv_mm(p, 1, g, jp)
                state["debt"] += 700
                fill()
                pv_evac_a(p, 0, g)
                pv_evac_a(p, 1, g)
                if p == 1:
                    # projection for this query group becomes available as
                    # soon as both normalizes land; it fills the next group.
                    # The muls must be EMITTED before proj units can pop
                    # (emission order is dependency order).
                    flush_mul()
                    for nf in range(8):
                        add_fill(600, "proj", (g, nf),
                                 (lambda g=g, nf=nf: proj_nf(g, nf)))

    with nc.named_scope("pair0"):
        run_pair(0, "p0")
    with nc.named_scope("pair1"):
        run_pair(1, "p1")

    with nc.named_scope("tail"):
        # flush remaining filler (late projections), with junk matmuls to
        # keep the HAM clock warm across the pv-evac dependency chain
        flush_mul()
        n = 0
        while filler:
            pop_one()
            n += 1
            if n % 3 == 0:
                junk_mm()
    if debug_out is not None:
        nc.sync.dma_start(out=debug_out["qkvT"], in_=qkvT)
        nc.sync.dma_start(
            out=debug_out["vnat"],
            in_=vnat.rearrange("p a b c d -> p (a b c d)"))
        nc.sync.dma_start(out=debug_out["yT"], in_=yT)
        if "b_t" in debug_out:
            nc.sync.dma_start(out=debug_out["b_t"], in_=b_t)
    ctx.close()


# ---------------------------------------------------------------------------
# host-side wrapper
# ---------------------------------------------------------------------------

_NC_CACHE = {}


def _get_nc():
    if "nc" not in _NC_CACHE:
        _NC_CACHE["nc"] = build_kernel()
    return _NC_CACHE["nc"]


def make_in_maps(x, W_attn, b_attn, W_proj, b_proj):
    B = x.shape[0]
    # multiplicative causal mask for the diagonal chunk, [k, q]: 1 where q >= k
    mask_np = np.triu(np.ones((128, 128), np.float32)).astype(bf16)
    ident_np = np.eye(128, dtype=np.float32).astype(bf16)
    in_maps = []
    for core in range(N_CORES):
        b = core // 4
        g = core % 4
        cols = np.r_[256 * g:256 * g + 256,
                     1024 + 256 * g:1024 + 256 * g + 256,
                     2048 + 256 * g:2048 + 256 * g + 256]
        # [C, 768] -> [6 nf, 128 p, 8 c, 128 n]: per-nf chunks contiguous so
        # each W DMA is one descriptor run per partition
        Wc = W_attn[:, cols].astype(bf16)
        Wc = np.ascontiguousarray(
            Wc.reshape(8, 128, 6, 128).transpose(2, 1, 0, 3))
        # x^T [C, T] -> [p, qb, c, t] so each per-qb DMA reads one contiguous
        # 8KB run per partition
        xb = x[b].T.astype(bf16).reshape(8, 128, 4, 512).transpose(1, 2, 0, 3)
        in_maps.append({
            "xT": np.ascontiguousarray(xb),
            "W": Wc,
            "bcols": np.ascontiguousarray(
                b_attn[cols[:256]].reshape(2, 128).T).astype(np.float32),
            "Wp": np.ascontiguousarray(
                W_proj[256 * g:256 * g + 256, :]).astype(bf16),
            "mask": mask_np,
            "ident": ident_np,
        })
    return in_maps


def kernel(x, W_attn, b_attn, W_proj, b_proj, _trace=False, _trace_kwargs=None):
    x = np.asarray(x, np.float32)
    W_attn = np.asarray(W_attn, np.float32)
    b_attn = np.asarray(b_attn, np.float32)
    W_proj = np.asarray(W_proj, np.float32)
    b_proj = np.asarray(b_proj, np.float32)

    nc = _get_nc()
    in_maps = make_in_maps(x, W_attn, b_attn, W_proj, b_proj)
    res = run_bass_kernel_spmd(
        nc, in_maps, core_ids=list(range(N_CORES)), trace=_trace,
        **(_trace_kwargs or {}),
    )
    B = x.shape[0]
    out = np.zeros((B, T, C), np.float32)
    for core in range(N_CORES):
        b = core // 4
        out[b] += res.results[core]["outT"].T
    # V bias folded through the projection + output bias (exact)
    out += (b_proj + b_attn[2 * C:3 * C] @ W_proj)[None, None, :]
    if _trace:
        kernel._last_results = res
    return out


if __name__ == "__main__":
    # smoke test: build only
    nc = build_kernel()
    print("built ok")
